# revision 22
# baseline (speedup 1.0000x reference)
"""Causal self-attention (B=4, T=2048, D=1024, H=16) on 8 TRN2 NeuronCores.

Sharding: 2D (batch x head-group). Core c handles batch b = c//2 and head
group g = c%2 (8 heads, processed as 4 pairs).

v2 layout strategy (per core):
  - x is passed pre-transposed from host: xT [D, T].
  - Q/K projections produce qT/kT [128 local dims, T] with head pair 2p/2p+1
    stacked on partitions 0-63 / 64-127; the 1/sqrt(dh) scale is folded into
    the Wq weights on the host.
  - V is projected directly into natural [token, dim] layout (lhsT = x chunk,
    rhs = WvT) -- no PE transposes -- stored bf16 with a ones column per head
    so the PV matmul also accumulates the softmax denominator.
  - Scores are computed transposed: S^T [keys, queries]; causal masking is an
    accumulating PE matmul that adds -38 to masked positions BEFORE exp
    (start/stop accumulation group), so no vector mask multiply is needed.
  - exp() runs without max-subtraction (scores ~N(0,1), fp32 exp safe);
    output P^T is bf16, feeding the PV matmul directly.
  - Softmax denominators: row 64 of the y PSUM tiles -> 1/d via the fast
    custom-DVE reciprocal (reciprocal_approx_fast), broadcast to 64
    partitions with two col-tiled K=1 matmuls, one multiply per head.
  - o_proj consumes ynorm (bf16) as stationary; Wo is bf16; each core emits
    a partial [T, D] product over its 512 local head dims; host sums the two
    partials per batch.

Emission order interleaves projection / attention / o_proj so the Tile
scheduler can fill TensorE gaps during ACT-bound attention stretches and
keep the PE HAM-warm.
"""

import os
import sys

import numpy as np

if not any(os.path.isdir(os.path.join(p, "concourse")) for p in sys.path):
    sys.path.insert(0, "/opt/trn_rl_repo")

import concourse.mybir as mybir
import concourse.tile as tile
from concourse import bacc
from concourse.bass_utils import run_bass_kernel_spmd

B, T, D, H, DH = 4, 2048, 1024, 16, 64
N_CORES = 8
GROUPS = 2          # head groups (tensor-parallel dim)
HPG = H // GROUPS   # heads per group/core
PAIRS = HPG // 2    # head pairs per core
NKB = T // 128      # 128-key blocks per batch
NQT = T // 512      # 512-query tiles per batch
VSTRIDE = NKB * 130 # vnat cols per pair: 16 blocks x [64 dims|1|64 dims|1]

F32 = mybir.dt.float32
I16 = mybir.dt.int16
# bf16 exp bit-trick: bits16(exp(x)) ~= x*128*log2(e) + 127*128 - 5.5
EXP_SCALE = 128.0 * 1.4426950408889634
EXP_BIAS = 127.0 * 128.0 - 5.5
F32R = mybir.dt.float32r
BF16 = mybir.dt.bfloat16


def build_nc():
    nc = bacc.Bacc("TRN2", target_bir_lowering=False, debug=False,
                   num_devices=N_CORES)
    xT = nc.dram_tensor("xT", [D, T], BF16, kind="ExternalInput").ap()
    wqR = nc.dram_tensor("wqR", [128, 4096], BF16, kind="ExternalInput").ap()
    wkR = nc.dram_tensor("wkR", [128, 4096], BF16, kind="ExternalInput").ap()
    wvT = nc.dram_tensor("wvT", [D, 512], BF16, kind="ExternalInput").ap()
    woT = nc.dram_tensor("woT", [512, D], BF16, kind="ExternalInput").ap()
    cpk = nc.dram_tensor("cpk", [128, 384], BF16, kind="ExternalInput").ap()
    onesb = nc.dram_tensor("onesb", [128, 128], BF16, kind="ExternalInput").ap()
    sel0 = nc.dram_tensor("sel0", [1, 128], F32R, kind="ExternalInput").ap()
    sel1 = nc.dram_tensor("sel1", [1, 128], F32R, kind="ExternalInput").ap()
    out = nc.dram_tensor("out", [T, D], F32, kind="ExternalOutput").ap()

    with tile.TileContext(nc) as tc:
        _body(tc, out, xT, wqR, wkR, wvT, woT, cpk, onesb, sel0, sel1)
    nc.compile()
    return nc


def _body(tc, out, xT, wqR, wkR, wvT, woT, cpk, onesb, sel0, sel1):
    nc = tc.nc
    from contextlib import ExitStack

    with ExitStack() as ctx:
        persist = ctx.enter_context(tc.tile_pool(name="persist", bufs=1))
        qT = persist.tile([128, PAIRS * T], BF16, tag="qT")
        kT = persist.tile([128, PAIRS * T], BF16, tag="kT")
        vnat = persist.tile([128, PAIRS * VSTRIDE], BF16, tag="vnat")
        ynorm = persist.tile([128, PAIRS * T], BF16, tag="ynorm")

        consts = ctx.enter_context(tc.tile_pool(name="consts", bufs=1))
        cpk_sb = consts.tile([128, 384], BF16, tag="cpk")
        nc.sync.dma_start(cpk_sb[:], cpk[:])
        mask2_sb = cpk_sb[:, 0:256]
        ident_sb = cpk_sb[:, 256:384]
        sel0_sb = consts.tile([1, 128], F32R, tag="sel0")
        sel1_sb = consts.tile([1, 128], F32R, tag="sel1")
        warm = consts.tile([128, 512], BF16, tag="warm")
        nc.vector.memset(warm[:], 0.0)

        wqkpool = ctx.enter_context(tc.tile_pool(name="wqk", bufs=1))
        wq_sb = wqkpool.tile([128, 4096], BF16, tag="wq")
        wk_sb = wqkpool.tile([128, 4096], BF16, tag="wk")
        wvpool = ctx.enter_context(tc.tile_pool(name="wv", bufs=1))
        wv_sb = wvpool.tile([128, 8 * 512], BF16, tag="wv")
        wopool = ctx.enter_context(tc.tile_pool(name="wo", bufs=1))
        wo_sb = []
        for p in range(PAIRS):
            wot = wopool.tile([128, 1024], BF16, tag=f"wo{p}")
            wo_sb.append(wot)

        xpool = ctx.enter_context(tc.tile_pool(name="xt", bufs=2))
        ppool = ctx.enter_context(tc.tile_pool(name="p", bufs=6))
        rpool = ctx.enter_context(tc.tile_pool(name="r", bufs=3))
        opool = ctx.enter_context(tc.tile_pool(name="osb", bufs=3))

        spool = ctx.enter_context(
            tc.tile_pool(name="s", bufs=2, space="PSUM"))
        ypool = ctx.enter_context(
            tc.tile_pool(name="y", bufs=1, space="PSUM"))
        shpool = ctx.enter_context(
            tc.tile_pool(name="sh", bufs=2, space="PSUM"))

        # ---------------- phase emitters -----------------------------
        x_sb = [None] * 8

        def load_x(half):
            for c in range(8):
                xt = xpool.tile([128, 1024], BF16, tag=f"x{c}")
                nc.sync.dma_start(
                    xt[:], xT[c * 128:(c + 1) * 128,
                              half * 1024:(half + 1) * 1024])
                x_sb[c] = xt

        def prologue_dmas():
            # warm-up burst: dummy matmuls while DMAs stream, so the PE
            # HAM un-throttles before the first real projection matmul
            wps = shpool.tile([128, 512], F32, tag="ps")
            for i in range(30):
                nc.tensor.matmul(wps[:], lhsT=warm[:, 0:128], rhs=warm[:],
                                 start=True, stop=True)
            # wq strips on sync queue, x strips on gpsimd queue: parallel
            # issue so the first q-proj matmul can start after ~0.8MB
            for c in range(8):
                nc.sync.dma_start(wq_sb[:, c * 512:(c + 1) * 512],
                                  wqR[:, c * 512:(c + 1) * 512])
                xt = xpool.tile([128, 1024], BF16, tag=f"x{c}")
                nc.gpsimd.dma_start(xt[:], xT[c * 128:(c + 1) * 128, 0:1024])
                x_sb[c] = xt
            for c in range(8):
                nc.sync.dma_start(wk_sb[:, c * 512:(c + 1) * 512],
                                  wkR[:, c * 512:(c + 1) * 512])
            ones_view = vnat[:].rearrange("r (p k m x) -> r (p k m) x",
                                          p=PAIRS, k=NKB, m=2)[:, :, 64:65]
            nc.sync.dma_start(ones_view.squeeze(), onesb[:])
            for c in range(8):
                nc.sync.dma_start(wv_sb[:, c * 512:(c + 1) * 512],
                                  wvT[c * 128:(c + 1) * 128, :])
            for p in range(PAIRS):
                nc.sync.dma_start(wo_sb[p][:], woT[p * 128:(p + 1) * 128, :])
            nc.gpsimd.dma_start(sel0_sb[:], sel0[:])
            nc.gpsimd.dma_start(sel1_sb[:], sel1[:])

        def proj_chunk_groups(half, sub):
            """Return per-group emitters for one 512-token chunk (8 q/k
            groups + 4 v groups), so they can interleave into an ACT-bound
            attention phase as PE filler."""
            groups = []
            for w_sb, dst in ((wq_sb, qT), (wk_sb, kT)):
                for p in range(PAIRS):
                    def g(w_sb=w_sb, dst=dst, p=p):
                        ps = shpool.tile([128, 512], F32, tag="ps")
                        for c in range(8):
                            nc.tensor.matmul(
                                ps[:],
                                lhsT=(w_sb[:, c * 512 + p * 128:
                                             c * 512 + (p + 1) * 128]),
                                rhs=(x_sb[c][:, sub * 512:(sub + 1) * 512]),
                                start=(c == 0), stop=(c == 7))
                        col0 = p * T + half * 1024 + sub * 512
                        nc.vector.tensor_copy(dst[:, col0:col0 + 512], ps[:])
                    groups.append(g)
            for tb in range(4):
                def g(tb=tb):
                    ps = shpool.tile([128, 512], F32, tag="ps")
                    tok0 = sub * 512 + tb * 128
                    for c in range(8):
                        nc.tensor.matmul(
                            ps[:],
                            lhsT=(x_sb[c][:, tok0:tok0 + 128]),
                            rhs=(wv_sb[:, c * 512:(c + 1) * 512]),
                            start=(c == 0), stop=(c == 7))
                    kb = half * 8 + sub * 4 + tb
                    srcv = ps[:].rearrange("r (p m x) -> r p m x",
                                           p=PAIRS, m=2)
                    dstv = vnat[:].rearrange(
                        "r (p k m x) -> r p k m x",
                        p=PAIRS, k=NKB, m=2)[:, :, kb:kb + 1, :, 0:64]
                    nc.vector.tensor_copy(dstv.squeeze(2), srcv)
                groups.append(g)
            # the closures read x_sb at EMISSION time; bind the tiles now
            xs = list(x_sb)
            def bindx(g):
                def h():
                    saved = list(x_sb)
                    x_sb[:] = xs
                    g()
                    x_sb[:] = saved
                return h
            return [bindx(g) for g in groups]

        def proj_chunk(half, sub):
            for g in proj_chunk_groups(half, sub):
                g()

        def pack_filler(*lists):
            """Merge filler emitters into 4 slots (one per attention pair)."""
            flat = [f for lst in lists for f in lst]
            k = (len(flat) + 3) // 4
            slots = []
            for i in range(0, len(flat), k):
                chunk = flat[i:i + k]
                slots.append(lambda chunk=chunk: [f() for f in chunk])
            return slots

        pending = [None]

        def _normalize(p, qt, y0, y1):
            den0 = rpool.tile([1, 512], F32R, tag="den0")
            den1 = rpool.tile([1, 512], F32R, tag="den1")
            nc.vector.tensor_copy(den0[:], y0[64:65, :])
            nc.vector.tensor_copy(den1[:], y1[64:65, :])
            # broadcast raw denominators to 64 partitions each (col-tiled
            # K=1 matmuls), then one fast-approx reciprocal on the full
            # [128, 512] tile (DVE cost is free-dim-bound, so this is as
            # cheap as a [1, 512] reciprocal)
            rbs = shpool.tile([128, 512], F32, tag="ps")
            nc.tensor.matmul(rbs[:], lhsT=sel0_sb[:],
                             rhs=den0[:], start=True, stop=False)
            nc.tensor.matmul(rbs[:], lhsT=sel1_sb[:],
                             rhs=den1[:], start=False, stop=True)
            rcp = rpool.tile([128, 512], F32, tag="rcp")
            nc.vector.reciprocal_approx_fast(out=rcp[:], in_=rbs[:])
            ycol = p * T + qt * 512
            nc.vector.tensor_mul(ynorm[0:64, ycol:ycol + 512],
                                 y0[0:64, :], rcp[0:64, :])
            nc.vector.tensor_mul(ynorm[64:128, ycol:ycol + 512],
                                 y1[0:64, :], rcp[64:128, :])

        def attn_qt(qt, filler=()):
            filler = list(filler)
            nkb = (qt + 1) * 4
            spread = max(2, (PAIRS * nkb) // (len(filler) + 1)) if filler \
                else 0
            it = 0
            for p in range(PAIRS):
                y0 = ypool.tile([65, 512], F32, tag="y0")
                y1 = ypool.tile([65, 512], F32, tag="y1")
                for kb in range(nkb):
                    o = kb - qt * 4
                    scol = max(0, o * 128)
                    width = 512 - scol
                    qcol = p * T + qt * 512 + scol
                    kcol = p * T + kb * 128
                    vbase = p * VSTRIDE + kb * 130
                    # both heads' scores in one 2-bank PSUM tile so a
                    # single ACT instruction exponentiates both
                    s01 = spool.tile([128, 1024], F32, tag="s01")
                    nc.tensor.matmul(
                        s01[:, 0:width],
                        lhsT=(kT[0:64, kcol:kcol + 128]),
                        rhs=(qT[0:64, qcol:qcol + width]),
                        start=True, stop=(o < 0))
                    nc.tensor.matmul(
                        s01[:, 512:512 + width],
                        lhsT=(kT[64:128, kcol:kcol + 128]),
                        rhs=(qT[64:128, qcol:qcol + width]),
                        start=True, stop=(o < 0))
                    if o >= 0:
                        # causal mask: accumulate -38 into masked positions
                        # of the diagonal 128-col chunk (both heads in one
                        # N=256 matmul: fp32r needs N>=256 for full rate)
                        mview = s01[:].rearrange("r (h x) -> r h x",
                                                 h=2)[:, :, 0:128]
                        nc.tensor.matmul(
                            mview, lhsT=ident_sb,
                            rhs=mask2_sb.rearrange("r (h x) -> r h x",
                                                      h=2),
                            start=False, stop=True)
                    p01 = ppool.tile([128, 1024], BF16, tag="p01")
                    sview = s01[:].rearrange("r (h x) -> r h x",
                                             h=2)[:, :, 0:width]
                    pview = p01[:].rearrange("r (h x) -> r h x",
                                             h=2)[:, :, 0:width]
                    nc.scalar.activation(
                        pview, sview, mybir.ActivationFunctionType.Exp)
                    nc.tensor.matmul(
                        y0[:, scol:512],
                        lhsT=(vnat[:, vbase:vbase + 65]),
                        rhs=(p01[:, 0:width]),
                        start=(kb == 0), stop=(kb == nkb - 1))
                    nc.tensor.matmul(
                        y1[:, scol:512],
                        lhsT=(vnat[:, vbase + 65:vbase + 130]),
                        rhs=(p01[:, 512:512 + width]),
                        start=(kb == 0), stop=(kb == nkb - 1))
                    if kb == 1 and pending[0] is not None:
                        pending[0]()
                        pending[0] = None
                    it += 1
                    if filler and kb >= 1 and it % spread == 0:
                        filler.pop(0)()
                if pending[0] is not None:
                    pending[0]()
                pending[0] = (lambda p=p, qt=qt, y0=y0, y1=y1:
                              _normalize(p, qt, y0, y1))
            for f in filler:
                f()

        def oproj_tt(tt):
            osb = opool.tile([128, 1024], F32, tag="osb")
            for n in range(2):
                ps = shpool.tile([128, 512], F32, tag="ps")
                for p in range(PAIRS):
                    nc.tensor.matmul(
                        ps[:],
                        lhsT=(ynorm[:, p * T + tt * 128:
                                      p * T + tt * 128 + 128]),
                        rhs=(wo_sb[p][:, n * 512:(n + 1) * 512]),
                        start=(p == 0), stop=(p == PAIRS - 1))
                nc.vector.tensor_copy(osb[:, n * 512:(n + 1) * 512],
                                      ps[:])
                nc.gpsimd.dma_start(
                    out[tt * 128:(tt + 1) * 128, n * 512:(n + 1) * 512],
                    osb[:, n * 512:(n + 1) * 512])

        def oproj_filler(qt):
            # flush the last pair's normalize so ynorm for qt is complete,
            # then hand back per-token-block emitters to interleave into the
            # NEXT attention phase (fills PE during its ACT-bound stalls)
            if pending[0] is not None:
                pending[0]()
                pending[0] = None
            return [lambda tt=tt: oproj_tt(tt)
                    for tt in range(qt * 4, qt * 4 + 4)]

        def oproj_qt(qt):
            for f in oproj_filler(qt):
                f()

        # ---------------- emission order -----------------------------
        prologue_dmas()
        proj_chunk(0, 0)
        attn_qt(0, filler=proj_chunk_groups(0, 1))
        load_x(1)
        attn_qt(1, filler=proj_chunk_groups(1, 0))
        attn_qt(2, filler=proj_chunk_groups(1, 1))
        # all of qt0-qt2's o_proj lands inside ACT-bound attn3 as PE filler
        attn_qt(3, filler=(oproj_filler(0) + oproj_filler(1) +
                           oproj_filler(2)))
        oproj_qt(3)


def shard_inputs(x, Wq, Wk, Wv, Wo):
    """Returns in_maps for cores 0..7 (core c: batch c//2, group c%2)."""
    import ml_dtypes
    x = np.ascontiguousarray(np.asarray(x, np.float32))
    mask1 = np.zeros((128, 128), np.float32)
    for r in range(128):
        mask1[r, :r] = -38.0  # S^T[key r, query j]: masked iff j < r
    mask = np.ascontiguousarray(np.concatenate([mask1, mask1], axis=1))
    ident = np.eye(128, dtype=np.float32)
    sel0 = np.zeros((1, 128), np.float32); sel0[0, :64] = 1.0
    sel1 = np.zeros((1, 128), np.float32); sel1[0, 64:] = 1.0
    in_maps = []
    perms = []
    for g in range(GROUPS):
        perm = np.array([(g * HPG + 2 * p + (q >= 64)) * 64 + (q % 64)
                         for p in range(PAIRS) for q in range(128)])
        perms.append(perm)
    w_cache = {}
    qscale = 1.0 / np.sqrt(DH)
    for g in range(GROUPS):
        perm = perms[g]
        wqT = (np.asarray(Wq, np.float32).T * qscale)[:, perm]
        wkT = np.asarray(Wk, np.float32).T[:, perm]
        # [r, c*512 + p*128 + o] = wT[c*128 + r, p*128 + o]
        def _re(wT):
            w4 = wT.reshape(8, 128, 4, 128)        # [c, r, p, o]
            return np.ascontiguousarray(
                w4.transpose(1, 0, 2, 3).reshape(128, 4096))
        w_cache[g] = {
            "wqR": _re(wqT).astype(ml_dtypes.bfloat16),
            "wkR": _re(wkT).astype(ml_dtypes.bfloat16),
            "wvT": np.ascontiguousarray(
                np.asarray(Wv, np.float32).T[:, perm]).astype(
                    ml_dtypes.bfloat16),
            "woT": np.ascontiguousarray(
                np.asarray(Wo, np.float32).T[perm, :]).astype(
                    ml_dtypes.bfloat16),
        }
    for c in range(N_CORES):
        b, g = c // 2, c % 2
        in_maps.append({
            "xT": np.ascontiguousarray(x[b].T).astype(ml_dtypes.bfloat16),
            "cpk": np.ascontiguousarray(
                np.concatenate([mask, ident], axis=1)).astype(
                    ml_dtypes.bfloat16),
            "onesb": np.ones((128, 128), ml_dtypes.bfloat16),
            "sel0": sel0, "sel1": sel1,
            **w_cache[g],
        })
    return in_maps


def kernel(x, Wq, Wk, Wv, Wo):
    nc = build_nc()
    in_maps = shard_inputs(x, Wq, Wk, Wv, Wo)
    res = run_bass_kernel_spmd(nc, in_maps, list(range(N_CORES)))
    out = np.empty((B, T, D), np.float32)
    for b in range(B):
        out[b] = res.results[2 * b]["out"] + res.results[2 * b + 1]["out"]
    return out


# revision 23
# speedup vs baseline: 1.0365x; 1.0365x over previous
"""Causal self-attention (B=4, T=2048, D=1024, H=16) on 8 TRN2 NeuronCores.

Sharding: 2D (batch x head-group). Core c handles batch b = c//2 and head
group g = c%2 (8 heads, processed as 4 pairs).

v2 layout strategy (per core):
  - x is passed pre-transposed from host: xT [D, T].
  - Q/K projections produce qT/kT [128 local dims, T] with head pair 2p/2p+1
    stacked on partitions 0-63 / 64-127; the 1/sqrt(dh) scale is folded into
    the Wq weights on the host.
  - V is projected directly into natural [token, dim] layout (lhsT = x chunk,
    rhs = WvT) -- no PE transposes -- stored bf16 with a ones column per head
    so the PV matmul also accumulates the softmax denominator.
  - Scores are computed transposed: S^T [keys, queries]; causal masking is an
    accumulating PE matmul that adds -38 to masked positions BEFORE exp
    (start/stop accumulation group), so no vector mask multiply is needed.
  - exp() runs without max-subtraction (scores ~N(0,1), fp32 exp safe);
    output P^T is bf16, feeding the PV matmul directly.
  - Softmax denominators: row 64 of the y PSUM tiles -> 1/d via the fast
    custom-DVE reciprocal (reciprocal_approx_fast), broadcast to 64
    partitions with two col-tiled K=1 matmuls, one multiply per head.
  - o_proj consumes ynorm (bf16) as stationary; Wo is bf16; each core emits
    a partial [T, D] product over its 512 local head dims; host sums the two
    partials per batch.

Emission order interleaves projection / attention / o_proj so the Tile
scheduler can fill TensorE gaps during ACT-bound attention stretches and
keep the PE HAM-warm.
"""

import os
import sys

import numpy as np

if not any(os.path.isdir(os.path.join(p, "concourse")) for p in sys.path):
    sys.path.insert(0, "/opt/trn_rl_repo")

import concourse.mybir as mybir
import concourse.tile as tile
from concourse import bacc
from concourse.bass_utils import run_bass_kernel_spmd

B, T, D, H, DH = 4, 2048, 1024, 16, 64
N_CORES = 8
GROUPS = 2          # head groups (tensor-parallel dim)
HPG = H // GROUPS   # heads per group/core
PAIRS = HPG // 2    # head pairs per core
NKB = T // 128      # 128-key blocks per batch
NQT = T // 512      # 512-query tiles per batch
VSTRIDE = NKB * 130 # vnat cols per pair: 16 blocks x [64 dims|1|64 dims|1]

F32 = mybir.dt.float32
I16 = mybir.dt.int16
# bf16 exp bit-trick: bits16(exp(x)) ~= x*128*log2(e) + 127*128 - 5.5
EXP_SCALE = 128.0 * 1.4426950408889634
EXP_BIAS = 127.0 * 128.0 - 5.5
F32R = mybir.dt.float32r
BF16 = mybir.dt.bfloat16


def build_nc():
    nc = bacc.Bacc("TRN2", target_bir_lowering=False, debug=False,
                   num_devices=N_CORES)
    xT = nc.dram_tensor("xT", [D, T], BF16, kind="ExternalInput").ap()
    wqR = nc.dram_tensor("wqR", [128, 4096], BF16, kind="ExternalInput").ap()
    wkR = nc.dram_tensor("wkR", [128, 4096], BF16, kind="ExternalInput").ap()
    wvT = nc.dram_tensor("wvT", [D, 512], BF16, kind="ExternalInput").ap()
    woT = nc.dram_tensor("woT", [512, D], BF16, kind="ExternalInput").ap()
    cpk = nc.dram_tensor("cpk", [128, 384], BF16, kind="ExternalInput").ap()
    onesb = nc.dram_tensor("onesb", [128, 128], BF16, kind="ExternalInput").ap()
    sel0 = nc.dram_tensor("sel0", [1, 128], F32R, kind="ExternalInput").ap()
    sel1 = nc.dram_tensor("sel1", [1, 128], F32R, kind="ExternalInput").ap()
    out = nc.dram_tensor("out", [T, D], F32, kind="ExternalOutput").ap()

    with tile.TileContext(nc) as tc:
        _body(tc, out, xT, wqR, wkR, wvT, woT, cpk, onesb, sel0, sel1)
    nc.compile()
    return nc


def _body(tc, out, xT, wqR, wkR, wvT, woT, cpk, onesb, sel0, sel1):
    nc = tc.nc
    from contextlib import ExitStack

    with ExitStack() as ctx:
        persist = ctx.enter_context(tc.tile_pool(name="persist", bufs=1))
        qT = persist.tile([128, PAIRS * T], BF16, tag="qT")
        kT = persist.tile([128, PAIRS * T], BF16, tag="kT")
        vnat = persist.tile([128, PAIRS * VSTRIDE], BF16, tag="vnat")
        ynorm = persist.tile([128, PAIRS * T], BF16, tag="ynorm")

        consts = ctx.enter_context(tc.tile_pool(name="consts", bufs=1))
        cpk_sb = consts.tile([128, 384], BF16, tag="cpk")
        nc.sync.dma_start(cpk_sb[:], cpk[:])
        mask2_sb = cpk_sb[:, 0:256]
        ident_sb = cpk_sb[:, 256:384]
        sel0_sb = consts.tile([1, 128], F32R, tag="sel0")
        sel1_sb = consts.tile([1, 128], F32R, tag="sel1")
        warm = consts.tile([128, 512], BF16, tag="warm")
        nc.vector.memset(warm[:], 0.0)

        wqkpool = ctx.enter_context(tc.tile_pool(name="wqk", bufs=1))
        wq_sb = wqkpool.tile([128, 4096], BF16, tag="wq")
        wk_sb = wqkpool.tile([128, 4096], BF16, tag="wk")
        wvpool = ctx.enter_context(tc.tile_pool(name="wv", bufs=1))
        wv_sb = wvpool.tile([128, 8 * 512], BF16, tag="wv")
        wopool = ctx.enter_context(tc.tile_pool(name="wo", bufs=1))
        wo_sb = []
        for p in range(PAIRS):
            wot = wopool.tile([128, 1024], BF16, tag=f"wo{p}")
            wo_sb.append(wot)

        xpool = ctx.enter_context(tc.tile_pool(name="xt", bufs=2))
        ppool = ctx.enter_context(tc.tile_pool(name="p", bufs=6))
        rpool = ctx.enter_context(tc.tile_pool(name="r", bufs=3))
        opool = ctx.enter_context(tc.tile_pool(name="osb", bufs=3))

        spool = ctx.enter_context(
            tc.tile_pool(name="s", bufs=2, space="PSUM"))
        ypool = ctx.enter_context(
            tc.tile_pool(name="y", bufs=1, space="PSUM"))
        shpool = ctx.enter_context(
            tc.tile_pool(name="sh", bufs=2, space="PSUM"))

        # ---------------- phase emitters -----------------------------
        x_sb = [None] * 8

        def load_x(half):
            for c in range(8):
                xt = xpool.tile([128, 1024], BF16, tag=f"x{c}")
                nc.sync.dma_start(
                    xt[:], xT[c * 128:(c + 1) * 128,
                              half * 1024:(half + 1) * 1024])
                x_sb[c] = xt

        def prologue_dmas():
            # warm-up burst: dummy matmuls while DMAs stream, so the PE
            # HAM un-throttles before the first real projection matmul
            wps = shpool.tile([128, 512], F32, tag="ps")
            for i in range(30):
                nc.tensor.matmul(wps[:], lhsT=warm[:, 0:128], rhs=warm[:],
                                 start=True, stop=True)
            # wq strips on sync queue, x strips on gpsimd queue: parallel
            # issue so the first q-proj matmul can start after ~0.8MB
            for c in range(8):
                nc.sync.dma_start(wq_sb[:, c * 512:(c + 1) * 512],
                                  wqR[:, c * 512:(c + 1) * 512])
                xt = xpool.tile([128, 1024], BF16, tag=f"x{c}")
                nc.gpsimd.dma_start(xt[:], xT[c * 128:(c + 1) * 128, 0:1024])
                x_sb[c] = xt
            for c in range(8):
                nc.sync.dma_start(wk_sb[:, c * 512:(c + 1) * 512],
                                  wkR[:, c * 512:(c + 1) * 512])
            ones_view = vnat[:].rearrange("r (p k m x) -> r (p k m) x",
                                          p=PAIRS, k=NKB, m=2)[:, :, 64:65]
            nc.sync.dma_start(ones_view.squeeze(), onesb[:])
            for c in range(8):
                nc.sync.dma_start(wv_sb[:, c * 512:(c + 1) * 512],
                                  wvT[c * 128:(c + 1) * 128, :])
            for p in range(PAIRS):
                nc.sync.dma_start(wo_sb[p][:], woT[p * 128:(p + 1) * 128, :])
            nc.gpsimd.dma_start(sel0_sb[:], sel0[:])
            nc.gpsimd.dma_start(sel1_sb[:], sel1[:])

        def proj_chunk_groups(half, sub):
            """Return per-group emitters for one 512-token chunk (8 q/k
            groups + 4 v groups), so they can interleave into an ACT-bound
            attention phase as PE filler."""
            groups = []
            for w_sb, dst in ((wq_sb, qT), (wk_sb, kT)):
                for p in range(PAIRS):
                    def g(w_sb=w_sb, dst=dst, p=p):
                        ps = shpool.tile([128, 512], F32, tag="ps")
                        for c in range(8):
                            nc.tensor.matmul(
                                ps[:],
                                lhsT=(w_sb[:, c * 512 + p * 128:
                                             c * 512 + (p + 1) * 128]),
                                rhs=(x_sb[c][:, sub * 512:(sub + 1) * 512]),
                                start=(c == 0), stop=(c == 7))
                        col0 = p * T + half * 1024 + sub * 512
                        nc.vector.tensor_copy(dst[:, col0:col0 + 512], ps[:])
                    groups.append(g)
            for tb in range(4):
                def g(tb=tb):
                    ps = shpool.tile([128, 512], F32, tag="ps")
                    tok0 = sub * 512 + tb * 128
                    for c in range(8):
                        nc.tensor.matmul(
                            ps[:],
                            lhsT=(x_sb[c][:, tok0:tok0 + 128]),
                            rhs=(wv_sb[:, c * 512:(c + 1) * 512]),
                            start=(c == 0), stop=(c == 7))
                    kb = half * 8 + sub * 4 + tb
                    srcv = ps[:].rearrange("r (p m x) -> r p m x",
                                           p=PAIRS, m=2)
                    dstv = vnat[:].rearrange(
                        "r (p k m x) -> r p k m x",
                        p=PAIRS, k=NKB, m=2)[:, :, kb:kb + 1, :, 0:64]
                    nc.vector.tensor_copy(dstv.squeeze(2), srcv)
                groups.append(g)
            # the closures read x_sb at EMISSION time; bind the tiles now
            xs = list(x_sb)
            def bindx(g):
                def h():
                    saved = list(x_sb)
                    x_sb[:] = xs
                    g()
                    x_sb[:] = saved
                return h
            return [bindx(g) for g in groups]

        def proj_chunk(half, sub):
            for g in proj_chunk_groups(half, sub):
                g()

        def pack_filler(*lists):
            """Merge filler emitters into 4 slots (one per attention pair)."""
            flat = [f for lst in lists for f in lst]
            k = (len(flat) + 3) // 4
            slots = []
            for i in range(0, len(flat), k):
                chunk = flat[i:i + k]
                slots.append(lambda chunk=chunk: [f() for f in chunk])
            return slots

        pending = [None]

        def _normalize(p, qt, y0, y1):
            den0 = rpool.tile([1, 512], F32R, tag="den0")
            den1 = rpool.tile([1, 512], F32R, tag="den1")
            nc.vector.tensor_copy(den0[:], y0[64:65, :])
            nc.vector.tensor_copy(den1[:], y1[64:65, :])
            # broadcast raw denominators to 64 partitions each (col-tiled
            # K=1 matmuls), then one fast-approx reciprocal on the full
            # [128, 512] tile (DVE cost is free-dim-bound, so this is as
            # cheap as a [1, 512] reciprocal)
            rbs = shpool.tile([128, 512], F32, tag="ps")
            nc.tensor.matmul(rbs[:], lhsT=sel0_sb[:],
                             rhs=den0[:], start=True, stop=False)
            nc.tensor.matmul(rbs[:], lhsT=sel1_sb[:],
                             rhs=den1[:], start=False, stop=True)
            rcp = rpool.tile([128, 512], F32, tag="rcp")
            nc.vector.reciprocal_approx_fast(out=rcp[:], in_=rbs[:])
            ycol = p * T + qt * 512
            nc.vector.tensor_mul(ynorm[0:64, ycol:ycol + 512],
                                 y0[0:64, :], rcp[0:64, :])
            nc.vector.tensor_mul(ynorm[64:128, ycol:ycol + 512],
                                 y1[0:64, :], rcp[64:128, :])

        def attn_qt(qt, filler=()):
            filler = list(filler)
            nkb = (qt + 1) * 4
            for p in range(PAIRS):
                if filler:
                    filler.pop(0)()
                y0 = ypool.tile([65, 512], F32, tag="y0")
                y1 = ypool.tile([65, 512], F32, tag="y1")
                for kb in range(nkb):
                    o = kb - qt * 4
                    scol = max(0, o * 128)
                    width = 512 - scol
                    qcol = p * T + qt * 512 + scol
                    kcol = p * T + kb * 128
                    vbase = p * VSTRIDE + kb * 130
                    # both heads' scores in one 2-bank PSUM tile so a
                    # single ACT instruction exponentiates both
                    s01 = spool.tile([128, 1024], F32, tag="s01")
                    nc.tensor.matmul(
                        s01[:, 0:width],
                        lhsT=(kT[0:64, kcol:kcol + 128]),
                        rhs=(qT[0:64, qcol:qcol + width]),
                        start=True, stop=(o < 0))
                    nc.tensor.matmul(
                        s01[:, 512:512 + width],
                        lhsT=(kT[64:128, kcol:kcol + 128]),
                        rhs=(qT[64:128, qcol:qcol + width]),
                        start=True, stop=(o < 0))
                    if o >= 0:
                        # causal mask: accumulate -38 into masked positions
                        # of the diagonal 128-col chunk (both heads in one
                        # N=256 matmul: fp32r needs N>=256 for full rate)
                        mview = s01[:].rearrange("r (h x) -> r h x",
                                                 h=2)[:, :, 0:128]
                        nc.tensor.matmul(
                            mview, lhsT=ident_sb,
                            rhs=mask2_sb.rearrange("r (h x) -> r h x",
                                                      h=2),
                            start=False, stop=True)
                    p01 = ppool.tile([128, 1024], BF16, tag="p01")
                    sview = s01[:].rearrange("r (h x) -> r h x",
                                             h=2)[:, :, 0:width]
                    pview = p01[:].rearrange("r (h x) -> r h x",
                                             h=2)[:, :, 0:width]
                    nc.scalar.activation(
                        pview, sview, mybir.ActivationFunctionType.Exp)
                    nc.tensor.matmul(
                        y0[:, scol:512],
                        lhsT=(vnat[:, vbase:vbase + 65]),
                        rhs=(p01[:, 0:width]),
                        start=(kb == 0), stop=(kb == nkb - 1))
                    nc.tensor.matmul(
                        y1[:, scol:512],
                        lhsT=(vnat[:, vbase + 65:vbase + 130]),
                        rhs=(p01[:, 512:512 + width]),
                        start=(kb == 0), stop=(kb == nkb - 1))
                    if kb == 1 and pending[0] is not None:
                        pending[0]()
                        pending[0] = None
                if pending[0] is not None:
                    pending[0]()
                pending[0] = (lambda p=p, qt=qt, y0=y0, y1=y1:
                              _normalize(p, qt, y0, y1))

        def oproj_tt(tt):
            osb = opool.tile([128, 1024], F32, tag="osb")
            for n in range(2):
                ps = shpool.tile([128, 512], F32, tag="ps")
                for p in range(PAIRS):
                    nc.tensor.matmul(
                        ps[:],
                        lhsT=(ynorm[:, p * T + tt * 128:
                                      p * T + tt * 128 + 128]),
                        rhs=(wo_sb[p][:, n * 512:(n + 1) * 512]),
                        start=(p == 0), stop=(p == PAIRS - 1))
                nc.vector.tensor_copy(osb[:, n * 512:(n + 1) * 512],
                                      ps[:])
                nc.gpsimd.dma_start(
                    out[tt * 128:(tt + 1) * 128, n * 512:(n + 1) * 512],
                    osb[:, n * 512:(n + 1) * 512])

        def oproj_filler(qt):
            # flush the last pair's normalize so ynorm for qt is complete,
            # then hand back per-token-block emitters to interleave into the
            # NEXT attention phase (fills PE during its ACT-bound stalls)
            if pending[0] is not None:
                pending[0]()
                pending[0] = None
            return [lambda tt=tt: oproj_tt(tt)
                    for tt in range(qt * 4, qt * 4 + 4)]

        def oproj_qt(qt):
            for f in oproj_filler(qt):
                f()

        # ---------------- emission order -----------------------------
        prologue_dmas()
        proj_chunk(0, 0)
        attn_qt(0, filler=pack_filler(proj_chunk_groups(0, 1)))
        load_x(1)
        attn_qt(1, filler=pack_filler(proj_chunk_groups(1, 0)))
        attn_qt(2, filler=pack_filler(proj_chunk_groups(1, 1)))
        # all of qt0-qt2's o_proj lands inside ACT-bound attn3 as PE filler
        attn_qt(3, filler=pack_filler(oproj_filler(0), oproj_filler(1),
                                      oproj_filler(2)))
        oproj_qt(3)


def shard_inputs(x, Wq, Wk, Wv, Wo):
    """Returns in_maps for cores 0..7 (core c: batch c//2, group c%2)."""
    import ml_dtypes
    x = np.ascontiguousarray(np.asarray(x, np.float32))
    mask1 = np.zeros((128, 128), np.float32)
    for r in range(128):
        mask1[r, :r] = -38.0  # S^T[key r, query j]: masked iff j < r
    mask = np.ascontiguousarray(np.concatenate([mask1, mask1], axis=1))
    ident = np.eye(128, dtype=np.float32)
    sel0 = np.zeros((1, 128), np.float32); sel0[0, :64] = 1.0
    sel1 = np.zeros((1, 128), np.float32); sel1[0, 64:] = 1.0
    in_maps = []
    perms = []
    for g in range(GROUPS):
        perm = np.array([(g * HPG + 2 * p + (q >= 64)) * 64 + (q % 64)
                         for p in range(PAIRS) for q in range(128)])
        perms.append(perm)
    w_cache = {}
    qscale = 1.0 / np.sqrt(DH)
    for g in range(GROUPS):
        perm = perms[g]
        wqT = (np.asarray(Wq, np.float32).T * qscale)[:, perm]
        wkT = np.asarray(Wk, np.float32).T[:, perm]
        # [r, c*512 + p*128 + o] = wT[c*128 + r, p*128 + o]
        def _re(wT):
            w4 = wT.reshape(8, 128, 4, 128)        # [c, r, p, o]
            return np.ascontiguousarray(
                w4.transpose(1, 0, 2, 3).reshape(128, 4096))
        w_cache[g] = {
            "wqR": _re(wqT).astype(ml_dtypes.bfloat16),
            "wkR": _re(wkT).astype(ml_dtypes.bfloat16),
            "wvT": np.ascontiguousarray(
                np.asarray(Wv, np.float32).T[:, perm]).astype(
                    ml_dtypes.bfloat16),
            "woT": np.ascontiguousarray(
                np.asarray(Wo, np.float32).T[perm, :]).astype(
                    ml_dtypes.bfloat16),
        }
    for c in range(N_CORES):
        b, g = c // 2, c % 2
        in_maps.append({
            "xT": np.ascontiguousarray(x[b].T).astype(ml_dtypes.bfloat16),
            "cpk": np.ascontiguousarray(
                np.concatenate([mask, ident], axis=1)).astype(
                    ml_dtypes.bfloat16),
            "onesb": np.ones((128, 128), ml_dtypes.bfloat16),
            "sel0": sel0, "sel1": sel1,
            **w_cache[g],
        })
    return in_maps


def kernel(x, Wq, Wk, Wv, Wo):
    nc = build_nc()
    in_maps = shard_inputs(x, Wq, Wk, Wv, Wo)
    res = run_bass_kernel_spmd(nc, in_maps, list(range(N_CORES)))
    out = np.empty((B, T, D), np.float32)
    for b in range(B):
        out[b] = res.results[2 * b]["out"] + res.results[2 * b + 1]["out"]
    return out
